# revision 1
# baseline (speedup 1.0000x reference)
"""Segment-max kernel for Trainium2 (8 NeuronCores, SPMD).

Strategy (data-parallel, per the sharding hint):
  - Shard embeddings/study_indexes along N across 8 cores (62500 rows each).
  - Host: per core, sort the shard's rows by segment id and lay them out
    feature-in-partition: partition p, column 256*t + 128*h + r holds
    sorted_row[128*t + r], feature 128*h + p.  Each 128-row tile is then a
    contiguous 256-column span whose per-feature max is a free-dim
    reduce_max — no on-device transpose needed.
  - Device: stream chunks at full HBM bandwidth; one VectorEngine
    reduce_max per chunk ([128, 2*tiles, 128] -> [128, 2*tiles]) produces
    per-tile per-feature maxes.
  - Host: pure tiles (single segment) combine via their device partials;
    the ~63 boundary tiles per core are re-reduced from the raw rows.
    Finally max across cores (the "all-reduce with max").
"""

import sys

sys.path.insert(0, "/opt/trn_rl_repo")

from contextlib import ExitStack

import numpy as np

import concourse.bacc as bacc
import concourse.bass as bass
import concourse.mybir as mybir

P = 128               # SBUF partitions
D = 256               # embedding dim
CHUNK_TILES = 16      # 128-row tiles per DMA chunk (2MB)
NBUF = 6              # chunk buffer depth
N_CORES = 8
RPB = P               # rows per partial block (one tile)

_NC_CACHE = {}


def build_nc(NT):
    """Bass program: NT 128-row tiles -> per-tile max partials.

    Inputs : emb   [128, NT*256] f32  (tile t = columns [256t, 256t+256);
                                       column 256t+128h+r = row r of the
                                       tile, feature 128h+p on partition p)
    Outputs: parts [128, 2*NT]   f32  (col 2t = max of tile t, features
                                       0-127 (feature in partition);
                                       col 2t+1 = features 128-255)
    """
    f32 = mybir.dt.float32
    chunk_sizes = []
    left = NT
    while left > 0:
        c = min(CHUNK_TILES, left)
        chunk_sizes.append(c)
        left -= c
    NCHUNK = len(chunk_sizes)

    nc = bacc.Bacc("TRN2")
    emb = nc.declare_dram_parameter("emb", [P, NT * D], f32, isOutput=False)
    parts = nc.declare_dram_parameter("parts", [P, 2 * NT], f32, isOutput=True)

    with (
        nc.Block() as block,
        nc.sbuf_tensor("partials", [P, 2 * NT], f32) as partials,
        nc.semaphore("st") as st,
        nc.semaphore("vr") as vr,
        ExitStack() as stack,
    ):
        chunks = [
            stack.enter_context(
                nc.sbuf_tensor(f"chunk{i}", [P, CHUNK_TILES * D], f32)
            )
            for i in range(NBUF)
        ]
        lds = [stack.enter_context(nc.semaphore(f"ld{i}")) for i in range(NBUF)]

        @block.sync
        def _(sync: bass.BassEngine):
            col = 0
            for c, csz in enumerate(chunk_sizes):
                if c >= NBUF:
                    # buffer c%NBUF free once chunk c-NBUF is reduced
                    sync.wait_ge(vr, c - NBUF + 1)
                sync.dma_start(
                    chunks[c % NBUF][:, : csz * D],
                    emb[:, col : col + csz * D],
                ).then_inc(lds[c % NBUF], 16)
                col += csz * D
            sync.wait_ge(vr, NCHUNK)
            sync.dma_start(parts[:], partials[:]).then_inc(st, 16)
            sync.wait_ge(st, 16)

        @block.vector
        def _(vector: bass.BassEngine):
            t0 = 0
            for c, csz in enumerate(chunk_sizes):
                b = c % NBUF
                vector.wait_ge(lds[b], 16 * (c // NBUF + 1))
                nc.vector.reduce_max(
                    partials[:, 2 * t0 : 2 * (t0 + csz)],
                    chunks[b][:, : csz * D].rearrange("p (k r) -> p k r", r=P),
                    axis=mybir.AxisListType.X,
                ).then_inc(vr, 1)
                t0 += csz

    nc.compile()
    return nc


def kernel(embeddings, study_indexes, num_segments):
    from concourse.bass_utils import run_bass_kernel_spmd

    emb = np.ascontiguousarray(np.asarray(embeddings, dtype=np.float32))
    idx = np.asarray(study_indexes).astype(np.int64)
    S = int(num_segments)
    N = emb.shape[0]
    Nc = N // N_CORES
    nt = -(-Nc // P)

    nc = _NC_CACHE.get(nt)
    if nc is None:
        nc = _NC_CACHE[nt] = build_nc(nt)

    plans = []
    in_maps = []
    for c in range(N_CORES):
        idx_c = idx[c * Nc : (c + 1) * Nc]
        shard = emb[c * Nc : (c + 1) * Nc]
        order = np.argsort(idx_c, kind="stable")
        rows = np.empty(nt * P, np.int64)
        rows[:Nc] = order
        rows[Nc:] = order[-1]                      # tail pad: repeat last row
        sorted_vals = shard[rows]                  # [nt*128, 256]
        # [p, t, h, r]: arr[p, 256t+128h+r] = sorted[128t+r, 128h+p]
        arr = (
            sorted_vals.reshape(nt, P, 2, P)
            .transpose(3, 0, 2, 1)
            .reshape(P, nt * D)
        )
        seg_sorted = idx_c[rows]
        blk_first = seg_sorted[0::RPB]             # [nt]
        blk_last = seg_sorted[RPB - 1 :: RPB]
        bnd_m = np.nonzero(blk_first != blk_last)[0]
        row_sel = (bnd_m[:, None] * RPB + np.arange(RPB)[None, :]).ravel()
        plans.append((seg_sorted, bnd_m, sorted_vals[row_sel]))
        del sorted_vals
        in_maps.append({"emb": np.ascontiguousarray(arr)})

    res = run_bass_kernel_spmd(nc, in_maps, list(range(N_CORES)))
    global _LAST_RESULT
    _LAST_RESULT = res

    out = np.full((S, D), -np.inf, dtype=np.float32)
    for c in range(N_CORES):
        parts = res.results[c]["parts"]            # [128, 2*nt] interleaved
        seg_sorted, bnd_m, bvals = plans[c]
        blk_first = seg_sorted[0::RPB]             # [nt]
        pure = np.ones(nt, bool)
        pure[bnd_m] = False

        # pure blocks: combine device partials by segment run
        pure_m = np.nonzero(pure)[0]
        if len(pure_m):
            psegs = blk_first[pure_m]
            starts = np.concatenate([[0], np.nonzero(np.diff(psegs))[0] + 1])
            p0 = parts[:, 2 * pure_m]              # [128, npure] feats 0-127
            p1 = parts[:, 2 * pure_m + 1]
            m0 = np.maximum.reduceat(p0, starts, axis=1)
            m1 = np.maximum.reduceat(p1, starts, axis=1)
            for j, s in enumerate(psegs[starts]):
                np.maximum(out[s, :P], m0[:, j], out=out[s, :P])
                np.maximum(out[s, P:], m1[:, j], out=out[s, P:])

        # boundary blocks: re-reduce from the raw (already sorted) rows
        if len(bnd_m):
            row_sel = (bnd_m[:, None] * RPB + np.arange(RPB)[None, :]).ravel()
            bsegs = seg_sorted[row_sel]            # sorted within and across runs
            starts = np.concatenate([[0], np.nonzero(np.diff(bsegs))[0] + 1])
            m = np.maximum.reduceat(bvals, starts, axis=0)
            for j, s in enumerate(bsegs[starts]):
                np.maximum(out[s], m[j], out=out[s])
    return out



# revision 2
# speedup vs baseline: 1.0461x; 1.0461x over previous
"""Segment-max kernel for Trainium2 (8 NeuronCores, SPMD) — v2.

Strategy (data-parallel, per the sharding hint):
  - Shard embeddings/study_indexes along N across 8 cores (62500 rows each).
  - Host: convert to bf16 (rel rounding err <=2^-8, far under the 2e-2
    gate) and stripe each segment's rows across CH chunks so that every
    accumulator cell (partition, column) holds a fixed (segment, feature,
    slot) across all chunks.  Column c = h*4096 + u: partition p carries
    feature 128h+p, u is the global slot id; segment s owns slots
    [U_s, U_s + K_s).
  - Device: stream CH 2MB bf16 chunks at full DMA bandwidth; one
    tensor_tensor(max) per chunk folds it into one of two interleaved
    accumulators (bf16 SBUF step-1 => 2x DVE perf mode, ~2x fewer vector
    cycles than tensor_reduce which only has a 1x uop).  Final merge, then
    DMA the 2MB accumulator back.
  - Host: per-segment max over the slot ranges, then max across cores
    (the "all-reduce with max").

vs v1 (per-tile reduce_max of f32 data): halves HBM traffic (the memory
roofline) and keeps the vector engine under the DMA time.
"""

import sys

sys.path.insert(0, "/opt/trn_rl_repo")

from contextlib import ExitStack

import numpy as np
import ml_dtypes

import concourse.bacc as bacc
import concourse.bass as bass
import concourse.mybir as mybir

P = 128               # SBUF partitions
D = 256               # embedding dim
NBUF = 7              # chunk buffer depth
N_CORES = 8
CH0 = 16              # default chunk count

_NC_CACHE = {}
_LAST_RESULT = None


def build_nc(CH, C):
    """Bass program: CH chunks of [128, C] bf16 -> elementwise running max.

    Inputs : emb   [128, CH*C] bf16  (chunk k = columns [k*C, (k+1)*C))
    Outputs: parts [128, C]    bf16  (elementwise max over the CH chunks)

    One accumulator, folded in column halves: consecutive vector ops touch
    disjoint halves, so no read-after-write pipeline drains and no final
    merge.  Each half streams out as soon as its last fold lands; the last
    chunk's DMA is split per half so the readback overlaps the tail.
    """
    bf16 = mybir.dt.bfloat16
    mx = mybir.AluOpType.max
    H = C // 2
    nc = bacc.Bacc("TRN2")
    emb = nc.declare_dram_parameter("emb", [P, CH * C], bf16, isOutput=False)
    parts = nc.declare_dram_parameter("parts", [P, C], bf16, isOutput=True)

    with (
        nc.Block() as block,
        nc.sbuf_tensor("acc", [P, C], bf16) as acc,
        nc.semaphore("st") as st,
        nc.semaphore("vr") as vr,
        nc.semaphore("mg") as mg,
        ExitStack() as stack,
    ):
        bufs = [
            stack.enter_context(nc.sbuf_tensor(f"chunk{i}", [P, C], bf16))
            for i in range(NBUF)
        ]
        lds = [stack.enter_context(nc.semaphore(f"ld{i}")) for i in range(NBUF)]

        Q = C // 4                            # tail quarter width

        @block.sync
        def _(sync: bass.BassEngine):
            for c in range(CH):
                b = c % NBUF
                if c >= NBUF:
                    # buffer b free once chunk c-NBUF is consumed
                    sync.wait_ge(vr, c - NBUF + 1)
                if c == CH - 1:
                    for q in range(4):
                        sync.dma_start(
                            bufs[b][:, q * Q : (q + 1) * Q],
                            emb[:, c * C + q * Q : c * C + (q + 1) * Q],
                        ).then_inc(lds[b], 16)
                else:
                    sync.dma_start(
                        bufs[b][:], emb[:, c * C : (c + 1) * C]
                    ).then_inc(lds[b], 16)
            for q in range(4):
                sync.wait_ge(mg, q + 1)
                sync.dma_start(
                    parts[:, q * Q : (q + 1) * Q], acc[:, q * Q : (q + 1) * Q]
                ).then_inc(st, 16)
            sync.wait_ge(st, 64)

        @block.vector
        def _(vector: bass.BassEngine):
            for c in range(CH - 1):
                b = c % NBUF
                base = 16 * (c // NBUF)
                vector.wait_ge(lds[b], base + 16)
                for lo, hi in [(0, H), (H, C)]:
                    if c == 0:
                        op = nc.vector.tensor_copy(acc[:, lo:hi], bufs[b][:, lo:hi])
                    else:
                        op = nc.vector.tensor_tensor(
                            acc[:, lo:hi], acc[:, lo:hi], bufs[b][:, lo:hi], mx
                        )
                    if hi == C:
                        op.then_inc(vr, 1)
            # last chunk: fold and release per column quarter
            c = CH - 1
            b = c % NBUF
            base = 16 * (c // NBUF)
            for q in range(4):
                vector.wait_ge(lds[b], base + 16 * (q + 1))
                nc.vector.tensor_tensor(
                    acc[:, q * Q : (q + 1) * Q],
                    acc[:, q * Q : (q + 1) * Q],
                    bufs[b][:, q * Q : (q + 1) * Q],
                    mx,
                ).then_inc(mg, 1)

    nc.compile()
    return nc


def _plan_core(idx_c, S, CH):
    """Slot plan for one core: stripe each segment's sorted rows across CH
    chunks.  Returns (order, counts, K, U, starts, total)."""
    order = np.argsort(idx_c, kind="stable")
    counts = np.bincount(idx_c, minlength=S)
    K = -(-counts // CH)                      # slots per segment
    U = np.concatenate([[0], np.cumsum(K)[:-1]])
    starts = np.concatenate([[0], np.cumsum(counts)[:-1]])
    return order, counts, K, U, starts, int(K.sum())


def kernel(embeddings, study_indexes, num_segments):
    from concourse.bass_utils import run_bass_kernel_spmd

    emb = np.asarray(embeddings, dtype=np.float32)
    idx = np.asarray(study_indexes).astype(np.int64)
    S = int(num_segments)
    N = emb.shape[0]
    Nc = N // N_CORES

    emb16 = emb.astype(ml_dtypes.bfloat16)

    CH = CH0
    while True:
        plans = [
            _plan_core(idx[c * Nc : (c + 1) * Nc], S, CH) for c in range(N_CORES)
        ]
        cap = max(p[5] for p in plans)        # max slots over cores
        if 2 * cap <= 16384:                  # accumulator <= 4MB
            break
        CH *= 2
    # slots per chunk (= columns per feature half), 64-col aligned
    CAP = -(-cap // 64) * 64
    C = 2 * CAP

    in_maps = []
    for c in range(N_CORES):
        order, counts, K, U, starts, total = plans[c]
        ROWS = np.full((CAP, CH), order[0], dtype=np.int64)
        if total:
            u_seg = np.repeat(np.arange(S), K)          # segment of each slot
            j_loc = np.arange(total) - np.repeat(U, K)  # slot idx within segment
            localpos = j_loc[:, None] * CH + np.arange(CH)[None, :]
            n_of = counts[u_seg][:, None]
            src = np.repeat(starts[u_seg][:, None], CH, 1) + localpos % n_of
            ROWS[:total] = order[src]
        shard16 = emb16[c * Nc : (c + 1) * Nc]
        arr = np.empty((P, CH * C), dtype=ml_dtypes.bfloat16)
        for k in range(CH):
            R = shard16[ROWS[:, k]]                      # [CAP, 256]
            arr[:, k * C : (k + 1) * C] = (
                R.reshape(CAP, 2, P).transpose(2, 1, 0).reshape(P, C)
            )
        in_maps.append({"emb": arr})

    nc = _NC_CACHE.get((CH, C))
    if nc is None:
        nc = _NC_CACHE[(CH, C)] = build_nc(CH, C)

    res = run_bass_kernel_spmd(nc, in_maps, list(range(N_CORES)))
    global _LAST_RESULT
    _LAST_RESULT = res

    out = np.full((S, D), -np.inf, dtype=np.float32)
    for c in range(N_CORES):
        order, counts, K, U, starts, total = plans[c]
        nz = counts > 0
        seg_nz = np.nonzero(nz)[0]
        if not len(seg_nz):
            continue
        parts = res.results[c]["parts"].astype(np.float32)  # [128, C]
        pf = parts.reshape(P, 2, CAP)[:, :, :total]
        m = np.maximum.reduceat(pf, U[nz], axis=2)          # [128, 2, n_nz]
        m = m.transpose(2, 1, 0).reshape(len(seg_nz), D)    # [n_nz, 256]
        out[seg_nz] = np.maximum(out[seg_nz], m)
    return out


# revision 3
# speedup vs baseline: 1.1009x; 1.0524x over previous
"""Segment-max kernel for Trainium2 (8 NeuronCores, SPMD) — v2.

Strategy (data-parallel, per the sharding hint):
  - Shard embeddings/study_indexes along N across 8 cores (62500 rows each).
  - Host: convert to bf16 (rel rounding err <=2^-8, far under the 2e-2
    gate) and stripe each segment's rows across CH chunks so that every
    accumulator cell (partition, column) holds a fixed (segment, feature,
    slot) across all chunks.  Column c = h*4096 + u: partition p carries
    feature 128h+p, u is the global slot id; segment s owns slots
    [U_s, U_s + K_s).
  - Device: stream CH 2MB bf16 chunks at full DMA bandwidth; one
    tensor_tensor(max) per chunk folds it into one of two interleaved
    accumulators (bf16 SBUF step-1 => 2x DVE perf mode, ~2x fewer vector
    cycles than tensor_reduce which only has a 1x uop).  Final merge, then
    DMA the 2MB accumulator back.
  - Host: per-segment max over the slot ranges, then max across cores
    (the "all-reduce with max").

vs v1 (per-tile reduce_max of f32 data): halves HBM traffic (the memory
roofline) and keeps the vector engine under the DMA time.
"""

import sys

sys.path.insert(0, "/opt/trn_rl_repo")

from contextlib import ExitStack

import numpy as np
import ml_dtypes

import concourse.bacc as bacc
import concourse.bass as bass
import concourse.mybir as mybir

P = 128               # SBUF partitions
D = 256               # embedding dim
NBUF = 9              # chunk buffer depth
N_CORES = 8
CH0 = 16              # default chunk count

_NC_CACHE = {}
_LAST_RESULT = None


def build_nc(CH, C):
    """Bass program: CH chunks of [128, C] bf16 -> elementwise running max.

    Inputs : emb   [128, CH*C] bf16  (chunk k = columns [k*C, (k+1)*C))
    Outputs: parts [128, C]    bf16  (elementwise max over the CH chunks)

    One accumulator, folded in column halves: consecutive vector ops touch
    disjoint halves, so no read-after-write pipeline drains and no final
    merge.  Each half streams out as soon as its last fold lands; the last
    chunk's DMA is split per half so the readback overlaps the tail.
    """
    bf16 = mybir.dt.bfloat16
    mx = mybir.AluOpType.max
    H = C // 2
    nc = bacc.Bacc("TRN2")
    emb = nc.declare_dram_parameter("emb", [P, CH * C], bf16, isOutput=False)
    parts = nc.declare_dram_parameter("parts", [P, C], bf16, isOutput=True)

    with (
        nc.Block() as block,
        nc.sbuf_tensor("acc", [P, C], bf16) as acc,
        nc.semaphore("st") as st,
        nc.semaphore("vr") as vr,
        nc.semaphore("mg") as mg,
        nc.semaphore("ai") as ai,
        ExitStack() as stack,
    ):
        bufs = [
            stack.enter_context(nc.sbuf_tensor(f"chunk{i}", [P, C], bf16))
            for i in range(NBUF)
        ]
        lds = [stack.enter_context(nc.semaphore(f"ld{i}")) for i in range(NBUF)]

        Q = C // 4                            # tail quarter width

        @block.sync
        def _(sync: bass.BassEngine):
            # chunk 0 initializes the accumulator directly, no fold needed
            sync.dma_start(acc[:], emb[:, 0:C]).then_inc(ai, 16)
            for c in range(1, CH):
                b = (c - 1) % NBUF
                if c > NBUF:
                    # buffer b free once chunk c-NBUF is folded
                    sync.wait_ge(vr, c - NBUF)
                if c == CH - 1:
                    for q in range(4):
                        sync.dma_start(
                            bufs[b][:, q * Q : (q + 1) * Q],
                            emb[:, c * C + q * Q : c * C + (q + 1) * Q],
                        ).then_inc(lds[b], 16)
                else:
                    sync.dma_start(
                        bufs[b][:], emb[:, c * C : (c + 1) * C]
                    ).then_inc(lds[b], 16)
            for q in range(4):
                sync.wait_ge(mg, q + 1)
                sync.dma_start(
                    parts[:, q * Q : (q + 1) * Q], acc[:, q * Q : (q + 1) * Q]
                ).then_inc(st, 16)
            sync.wait_ge(st, 64)

        @block.vector
        def _(vector: bass.BassEngine):
            vector.wait_ge(ai, 16)
            for c in range(1, CH - 1):
                b = (c - 1) % NBUF
                base = 16 * ((c - 1) // NBUF)
                vector.wait_ge(lds[b], base + 16)
                for lo, hi in [(0, H), (H, C)]:
                    op = nc.vector.tensor_tensor(
                        acc[:, lo:hi], acc[:, lo:hi], bufs[b][:, lo:hi], mx
                    )
                    if hi == C:
                        op.then_inc(vr, 1)
            # last chunk: fold and release per column quarter
            c = CH - 1
            b = (c - 1) % NBUF
            base = 16 * ((c - 1) // NBUF)
            for q in range(4):
                vector.wait_ge(lds[b], base + 16 * (q + 1))
                nc.vector.tensor_tensor(
                    acc[:, q * Q : (q + 1) * Q],
                    acc[:, q * Q : (q + 1) * Q],
                    bufs[b][:, q * Q : (q + 1) * Q],
                    mx,
                ).then_inc(mg, 1)

    nc.compile()
    return nc


def _plan_core(idx_c, S, CH):
    """Slot plan for one core: stripe each segment's sorted rows across CH
    chunks.  Returns (order, counts, K, U, starts, total)."""
    order = np.argsort(idx_c, kind="stable")
    counts = np.bincount(idx_c, minlength=S)
    K = -(-counts // CH)                      # slots per segment
    U = np.concatenate([[0], np.cumsum(K)[:-1]])
    starts = np.concatenate([[0], np.cumsum(counts)[:-1]])
    return order, counts, K, U, starts, int(K.sum())


def kernel(embeddings, study_indexes, num_segments):
    from concourse.bass_utils import run_bass_kernel_spmd

    emb = np.asarray(embeddings, dtype=np.float32)
    idx = np.asarray(study_indexes).astype(np.int64)
    S = int(num_segments)
    N = emb.shape[0]
    Nc = N // N_CORES

    emb16 = emb.astype(ml_dtypes.bfloat16)

    CH = CH0
    while True:
        plans = [
            _plan_core(idx[c * Nc : (c + 1) * Nc], S, CH) for c in range(N_CORES)
        ]
        cap = max(p[5] for p in plans)        # max slots over cores
        if 2 * cap <= 16384:                  # accumulator <= 4MB
            break
        CH *= 2
    # slots per chunk (= columns per feature half), 64-col aligned
    CAP = -(-cap // 64) * 64
    C = 2 * CAP

    in_maps = []
    for c in range(N_CORES):
        order, counts, K, U, starts, total = plans[c]
        ROWS = np.full((CAP, CH), order[0], dtype=np.int64)
        if total:
            u_seg = np.repeat(np.arange(S), K)          # segment of each slot
            j_loc = np.arange(total) - np.repeat(U, K)  # slot idx within segment
            localpos = j_loc[:, None] * CH + np.arange(CH)[None, :]
            n_of = counts[u_seg][:, None]
            src = np.repeat(starts[u_seg][:, None], CH, 1) + localpos % n_of
            ROWS[:total] = order[src]
        shard16 = emb16[c * Nc : (c + 1) * Nc]
        arr = np.empty((P, CH * C), dtype=ml_dtypes.bfloat16)
        for k in range(CH):
            R = shard16[ROWS[:, k]]                      # [CAP, 256]
            arr[:, k * C : (k + 1) * C] = (
                R.reshape(CAP, 2, P).transpose(2, 1, 0).reshape(P, C)
            )
        in_maps.append({"emb": arr})

    nc = _NC_CACHE.get((CH, C))
    if nc is None:
        nc = _NC_CACHE[(CH, C)] = build_nc(CH, C)

    res = run_bass_kernel_spmd(nc, in_maps, list(range(N_CORES)))
    global _LAST_RESULT
    _LAST_RESULT = res

    out = np.full((S, D), -np.inf, dtype=np.float32)
    for c in range(N_CORES):
        order, counts, K, U, starts, total = plans[c]
        nz = counts > 0
        seg_nz = np.nonzero(nz)[0]
        if not len(seg_nz):
            continue
        parts = res.results[c]["parts"].astype(np.float32)  # [128, C]
        pf = parts.reshape(P, 2, CAP)[:, :, :total]
        m = np.maximum.reduceat(pf, U[nz], axis=2)          # [128, 2, n_nz]
        m = m.transpose(2, 1, 0).reshape(len(seg_nz), D)    # [n_nz, 256]
        out[seg_nz] = np.maximum(out[seg_nz], m)
    return out
